# revision 1
# baseline (speedup 1.0000x reference)
"""DGAD net (vq_codebook) kernel for 8x Trainium2 NeuronCores.

Contract: kernel(**inputs) takes the FULL unsharded inputs (numpy, keyed as in
setup_inputs) and returns the FULL [4,1] float32 output. Inside, the batch
(128) is sharded 16-per-core across 8 cores (data parallel); weights are
replicated. Each core emits [1,4] partial sums (ce, origin_svdd, class_svdd,
align); the final all-reduce (sum across 8 cores, /128) happens on host during
the unshard step.

Key algebraic move: mean_HW(einsum('bchw,oc->bohw', x, w)) ==
einsum('bc,oc->bo', mean_HW(x), w) — the 2e11-FLOP 1x1 conv collapses to
pooling x_mid (205MB of streaming, the real cost) plus a tiny matmul.

Per-core pipeline:
  - x_deep [16,2048,49] pooled over HW with one 6.4MB DMA + one DVE reduce.
    The flat [16, 100352] view is split p=128 x 784 so DMA runs are 3136B
    contiguous; feature d lives at (partition p, col j) with d = 16p + j, and
    ow1's rows are pre-permuted on host to match.
  - x_mid [16,512,784] streamed in 8 3.2MB chunks (c on partitions, HW
    contiguous), DVE reduce_sum per chunk.
  - MLP chains on the PE in bf16 (host pre-transposes + casts weights; fp32
    PSUM accumulation). leaky_relu = max(x, 0.01x) via ACT mul + DVE max.
  - VQ tail in fp32: sim[b,k] = |t|^2 - 2 t.p + |p|^2 via one augmented
    matmul; argmax via is_ge(sim, rowmax) one-hot; proto gather via one-hot
    matmul; CE = log(sum exp(sim - max)); svdd partition-reductions via
    ones-vector matmuls.
"""

import numpy as np
import ml_dtypes

N_CORES = 8
B = 128
BC = B // N_CORES  # 16 samples per core

BF = ml_dtypes.bfloat16

_CACHE = {}


def _build_program():
    import concourse.bass as bass  # noqa: F401
    import concourse.mybir as mybir
    import concourse.tile as tile
    from concourse import bacc
    from contextlib import ExitStack

    dt = mybir.dt
    AF = mybir.ActivationFunctionType
    ALU = mybir.AluOpType
    AX = mybir.AxisListType
    f32, bf16 = dt.float32, dt.bfloat16

    nc = bacc.Bacc("TRN2", target_bir_lowering=False, debug=False,
                   enable_asserts=True, num_devices=N_CORES)

    def din(name, shape, d):
        return nc.dram_tensor(name, shape, d, kind="ExternalInput").ap()

    xm = din("xm", [BC, 512, 784], f32)
    xd = din("xd", [BC, 100352], f32)
    wshT_d = din("wshT", [512, 2048], bf16)
    ow1T_d = din("ow1T", [2048, 1024], bf16)  # rows pre-permuted: row j*128+p = ow1.T row 16p+j
    sw1T_d = din("sw1T", [2048, 1024], bf16)
    ow2T_d = din("ow2T", [1024, 512], bf16)
    sw2T_d = din("sw2T", [1024, 512], bf16)
    ow3T_d = din("ow3T", [512, 64], bf16)
    sw3T_d = din("sw3T", [512, 64], bf16)
    tw1T_d = din("tw1T", [128, 64], f32)
    tw2T_d = din("tw2T", [64, 64], f32)
    cw1T_d = din("cw1T", [128, 64], f32)
    cw2T_d = din("cw2T", [64, 64], f32)
    qw1T_d = din("qw1T", [64, 64], f32)
    qw2T_d = din("qw2T", [64, 64], f32)
    protoT_d = din("protoT", [64, 4], f32)
    proto_pad_d = din("proto_pad", [4, 128], f32)
    center_pad_d = din("center_pad", [1, 128], f32)
    center_col_d = din("center_col", [64, 1], f32)
    catid_d = din("catid", [64, 128], f32)
    id16_d = din("id16", [16, 16], f32)
    out_d = nc.dram_tensor("out", [1, 4], f32, kind="ExternalOutput").ap()

    with tile.TileContext(nc) as tc, ExitStack() as ctx:
        wp = ctx.enter_context(tc.tile_pool(name="wp", bufs=1))
        xp = ctx.enter_context(tc.tile_pool(name="xp", bufs=2))
        dp = ctx.enter_context(tc.tile_pool(name="dp", bufs=1))
        ap = ctx.enter_context(tc.tile_pool(name="ap", bufs=1))
        pp = ctx.enter_context(tc.tile_pool(name="pp", bufs=4, space="PSUM"))
        pt = ctx.enter_context(tc.tile_pool(name="pt", bufs=3, space="PSUM"))

        # ---------- x_deep: one 6.4MB DMA (3136B-contiguous runs) ----------
        xd_t = dp.tile([128, 16, 16, 49], f32, tag="xd")
        nc.sync.dma_start(out=xd_t[:],
                          in_=xd.rearrange("b (p j h) -> p b j h", p=128, j=16, h=49))

        ow1_t = wp.tile([128, 16, 1024], bf16, tag="ow1")
        nc.sync.dma_start(out=ow1_t[:], in_=ow1T_d.rearrange("(j p) o -> p j o", p=128))

        # pooled x_deep: xdsum[p, b, j] = sum_h xd[b, 16p+j, h]
        xdsum = ap.tile([128, 16, 16], f32, tag="xdsum")
        nc.vector.reduce_sum(xdsum[:], xd_t[:], axis=AX.X)
        xdb = ap.tile([128, 16, 16], bf16, tag="xdb")
        nc.vector.tensor_scalar(xdb[:], xdsum[:], 1.0 / 49.0, None, op0=ALU.mult)

        # ---------- weight/const tiles (DMAs interleaved with xm stream) ----------
        wsh_t = wp.tile([128, 4, 2048], bf16, tag="wsh")
        sw1_t = wp.tile([128, 16, 1024], bf16, tag="sw1")
        ow2_t = wp.tile([128, 8, 512], bf16, tag="ow2")
        sw2_t = wp.tile([128, 8, 512], bf16, tag="sw2")
        ow3_t = wp.tile([128, 4, 64], bf16, tag="ow3")
        sw3_t = wp.tile([128, 4, 64], bf16, tag="sw3")
        tw1_t = wp.tile([128, 64], f32, tag="tw1")
        tw2_t = wp.tile([64, 64], f32, tag="tw2")
        cw1_t = wp.tile([128, 64], f32, tag="cw1")
        cw2_t = wp.tile([64, 64], f32, tag="cw2")
        qw1_t = wp.tile([64, 64], f32, tag="qw1")
        qw2_t = wp.tile([64, 64], f32, tag="qw2")
        protoT_t = wp.tile([64, 4], f32, tag="protoT")
        proto_pad_t = wp.tile([4, 128], f32, tag="proto_pad")
        center_pad_t = wp.tile([1, 128], f32, tag="center_pad")
        center_col_t = wp.tile([64, 1], f32, tag="center_col")
        catid_t = wp.tile([64, 128], f32, tag="catid")
        id16_t = wp.tile([16, 16], f32, tag="id16")

        def small_dmas():
            nc.sync.dma_start(out=ow2_t[:], in_=ow2T_d.rearrange("(k p) o -> p k o", p=128))
            nc.sync.dma_start(out=sw2_t[:], in_=sw2T_d.rearrange("(k p) o -> p k o", p=128))
            nc.sync.dma_start(out=ow3_t[:], in_=ow3T_d.rearrange("(k p) o -> p k o", p=128))
            nc.sync.dma_start(out=sw3_t[:], in_=sw3T_d.rearrange("(k p) o -> p k o", p=128))
            for t_, d_ in ((tw1_t, tw1T_d), (tw2_t, tw2T_d), (cw1_t, cw1T_d),
                           (cw2_t, cw2T_d), (qw1_t, qw1T_d), (qw2_t, qw2T_d),
                           (protoT_t, protoT_d), (proto_pad_t, proto_pad_d),
                           (center_pad_t, center_pad_d), (center_col_t, center_col_d),
                           (catid_t, catid_d), (id16_t, id16_d)):
                nc.sync.dma_start(out=t_[:], in_=d_)

        # ---------- x_mid stream: 4 c-chunks x 2 half-batches of 8 ----------
        xmsum = ap.tile([128, 4, 16], f32, tag="xmsum")
        for h in range(8):
            cc, half = divmod(h, 2)
            b0 = 8 * half
            t = xp.tile([128, 8, 784], f32, tag="xmt")
            nc.sync.dma_start(
                out=t[:],
                in_=xm[b0:b0 + 8, cc * 128:(cc + 1) * 128, :].rearrange("b c h -> c b h"))
            nc.vector.reduce_sum(xmsum[:, cc, b0:b0 + 8], t[:], axis=AX.X)
            if h == 0:
                nc.sync.dma_start(out=wsh_t[:], in_=wshT_d.rearrange("(c p) o -> p c o", p=128))
            elif h == 1:
                nc.sync.dma_start(out=sw1_t[:], in_=sw1T_d.rearrange("(k p) o -> p k o", p=128))
            elif h == 2:
                small_dmas()

        # leaky relu: dst = max(ps, 0.01*ps)
        def leaky(dst, ps_in, np_, tagn):
            t_ = ap.tile([128, 16], f32, tag="lk_" + tagn)
            nc.scalar.mul(t_[:np_, :], ps_in, 0.01)
            nc.vector.tensor_tensor(dst, ps_in, t_[:np_, :], op=ALU.max)

        # chain layer: dst_m = act(sum_k W[:, k, m*msz:(m+1)*msz].T @ rhs_k)
        def layer(w_t, n_k, n_m, m_sz, rhs_fn, dst_fn, tagn, act=True):
            for m in range(n_m):
                ps = pp.tile([128, 16], f32, tag="mm")
                for k in range(n_k):
                    nc.tensor.matmul(ps[:m_sz, :], w_t[:, k, m * m_sz:(m + 1) * m_sz],
                                     rhs_fn(k), start=(k == 0), stop=(k == n_k - 1))
                if act:
                    leaky(dst_fn(m), ps[:m_sz, :], m_sz, tagn)
                else:
                    nc.scalar.copy(dst_fn(m), ps[:m_sz, :])

        # ---------- origin chain (runs during x_mid stream) ----------
        y1o = ap.tile([128, 8, 16], bf16, tag="y1o")
        layer(ow1_t, 16, 8, 128, lambda k: xdb[:, :, k], lambda m: y1o[:, m, :], "o1")
        y2o = ap.tile([128, 4, 16], bf16, tag="y2o")
        layer(ow2_t, 8, 4, 128, lambda k: y1o[:, k, :], lambda m: y2o[:, m, :], "o2")
        origin = ap.tile([64, 16], f32, tag="origin")
        layer(ow3_t, 4, 1, 64, lambda k: y2o[:, k, :], lambda m: origin[:], "o3")

        # ---------- qw chain + origin_svdd (needs only origin) ----------
        ones64 = ap.tile([64, 1], f32, tag="ones64")
        nc.gpsimd.memset(ones64[:], 1.0)
        ones16 = ap.tile([16, 1], f32, tag="ones16")
        nc.gpsimd.memset(ones16[:], 1.0)
        ones1x16 = ap.tile([1, 16], f32, tag="ones1x16")
        nc.gpsimd.memset(ones1x16[:], 1.0)

        def small_mlp(wa, wb, rhs, dst, tagn):
            psa = pt.tile([128, 16], f32, tag="tail")
            nc.tensor.matmul(psa[:64, :], wa[:, 0:64], rhs, start=True, stop=True)
            mid = ap.tile([64, 16], f32, tag="mid_" + tagn)
            leaky(mid[:], psa[:64, :], 64, "sa_" + tagn)
            psb = pt.tile([128, 16], f32, tag="tail")
            nc.tensor.matmul(psb[:64, :], wb[:, 0:64], mid[:], start=True, stop=True)
            leaky(dst, psb[:64, :], 64, "sb_" + tagn)

        def svdd(feat, dst_sb, tagn):
            # dst_sb[1,16] = sum_f (feat - center)^2  (partition reduce via ones matmul)
            d_ = ap.tile([64, 16], f32, tag="d_" + tagn)
            nc.vector.tensor_scalar(d_[:], feat, center_col_t[:, 0:1], None, op0=ALU.subtract)
            sq = ap.tile([64, 16], f32, tag="sq_" + tagn)
            nc.vector.tensor_tensor(sq[:], d_[:], d_[:], op=ALU.mult)
            psv = pt.tile([128, 16], f32, tag="tail")
            nc.tensor.matmul(psv[0:1, :], ones64[:], sq[:], start=True, stop=True)
            nc.scalar.copy(dst_sb, psv[0:1, :])

        qf = ap.tile([64, 16], f32, tag="qf")
        small_mlp(qw1_t, qw2_t, origin[:], qf[:], "q")
        osvdd = ap.tile([1, 16], f32, tag="osvdd")
        svdd(qf[:], osvdd[:], "o")

        # ---------- shallow pool matmul (after all x_mid chunks) ----------
        xmb = ap.tile([128, 4, 16], bf16, tag="xmb")
        nc.vector.tensor_scalar(xmb[:], xmsum[:], 1.0 / 784.0, None, op0=ALU.mult)
        spb = ap.tile([128, 16, 16], bf16, tag="spb")
        layer(wsh_t, 4, 16, 128, lambda k: xmb[:, k, :], lambda m: spb[:, m, :], "sp",
              act=False)

        # ---------- shallow chain ----------
        y1s = ap.tile([128, 8, 16], bf16, tag="y1s")
        layer(sw1_t, 16, 8, 128, lambda k: spb[:, k, :], lambda m: y1s[:, m, :], "s1")
        y2s = ap.tile([128, 4, 16], bf16, tag="y2s")
        layer(sw2_t, 8, 4, 128, lambda k: y1s[:, k, :], lambda m: y2s[:, m, :], "s2")
        shallow = ap.tile([64, 16], f32, tag="shallow")
        layer(sw3_t, 4, 1, 64, lambda k: y2s[:, k, :], lambda m: shallow[:], "s3")

        # ---------- texture = mlp(cat[shallow, shallow - center]) ----------
        neg_center_pad = ap.tile([1, 128], f32, tag="ncp")
        nc.vector.tensor_scalar(neg_center_pad[:], center_pad_t[:], -1.0, None, op0=ALU.mult)
        cat1_ps = pt.tile([128, 16], f32, tag="tail")
        nc.tensor.matmul(cat1_ps[:], catid_t[:], shallow[:], start=True, stop=False)
        nc.tensor.matmul(cat1_ps[:], neg_center_pad[:], ones1x16[:], start=False, stop=True)
        cat1 = ap.tile([128, 16], f32, tag="cat1")
        nc.scalar.copy(cat1[:], cat1_ps[:])

        t1ps = pt.tile([128, 16], f32, tag="tail")
        nc.tensor.matmul(t1ps[:64, :], tw1_t[:, 0:64], cat1[:], start=True, stop=True)
        t1 = ap.tile([64, 16], f32, tag="t1")
        leaky(t1[:], t1ps[:64, :], 64, "t1")
        # texture lands in rows 0..63 of the 65-row sim lhsT
        sim_lhs = ap.tile([65, 16], f32, tag="sim_lhs")
        t2ps = pt.tile([128, 16], f32, tag="tail")
        nc.tensor.matmul(t2ps[:64, :], tw2_t[:, 0:64], t1[:], start=True, stop=True)
        leaky(sim_lhs[0:64, :], t2ps[:64, :], 64, "tx")

        # ---------- sim[b,k] = |t_b|^2 - 2 t.p_k + |p_k|^2 (augmented matmul) ----------
        t2 = ap.tile([64, 16], f32, tag="t2")
        nc.vector.tensor_tensor(t2[:], sim_lhs[0:64, :], sim_lhs[0:64, :], op=ALU.mult)
        tsq_ps = pt.tile([128, 16], f32, tag="tail")
        nc.tensor.matmul(tsq_ps[0:1, :], ones64[:], t2[:], start=True, stop=True)
        # |t|^2 row -> sim_lhs row 64 (partition-shift copy PSUM p0 -> SBUF p64)
        nc.scalar.copy(sim_lhs[64:65, :], tsq_ps[0:1, :])

        rhs_sim = ap.tile([65, 4], f32, tag="rhs_sim")
        nc.vector.tensor_scalar(rhs_sim[0:64, :], protoT_t[:], -2.0, None, op0=ALU.mult)
        nc.gpsimd.memset(rhs_sim[64:65, :], 1.0)

        pT2 = ap.tile([64, 4], f32, tag="pT2")
        nc.vector.tensor_tensor(pT2[:], protoT_t[:], protoT_t[:], op=ALU.mult)
        pn_ps = pt.tile([128, 16], f32, tag="tail")
        nc.tensor.matmul(pn_ps[0:1, 0:4], ones64[:], pT2[:], start=True, stop=True)
        pnorm = ap.tile([1, 4], f32, tag="pnorm")
        nc.scalar.copy(pnorm[:], pn_ps[0:1, 0:4])

        sim_ps = pt.tile([128, 16], f32, tag="tail")
        nc.tensor.matmul(sim_ps[0:16, 0:4], sim_lhs[:], rhs_sim[:], start=True, stop=False)
        nc.tensor.matmul(sim_ps[0:16, 0:4], ones1x16[:], pnorm[:], start=False, stop=True)
        sim_sb = ap.tile([16, 4], f32, tag="sim_sb")
        nc.vector.tensor_copy(sim_sb[:], sim_ps[0:16, 0:4])

        # ---------- CE + argmax one-hot ----------
        m16 = ap.tile([16, 1], f32, tag="m16")
        nc.vector.reduce_max(m16[:], sim_sb[:], axis=AX.X)
        negm = ap.tile([16, 1], f32, tag="negm")
        nc.vector.tensor_scalar(negm[:], m16[:], -1.0, None, op0=ALU.mult)
        e_t = ap.tile([16, 4], f32, tag="e_t")
        s16 = ap.tile([16, 1], f32, tag="s16")
        nc.scalar.activation(e_t[:], sim_sb[:], AF.Exp, bias=negm[:, 0:1], accum_out=s16[:])
        ce_col = ap.tile([16, 1], f32, tag="ce_col")
        nc.scalar.activation(ce_col[:], s16[:], AF.Ln)

        onehotT = ap.tile([16, 4], f32, tag="onehotT")
        nc.vector.tensor_scalar(onehotT[:], sim_sb[:], m16[:, 0:1], None, op0=ALU.is_ge)
        oh_ps = pt.tile([128, 16], f32, tag="tail")
        nc.tensor.transpose(oh_ps[0:4, 0:16], onehotT[:], id16_t[:])
        oh_sb = ap.tile([4, 16], f32, tag="oh_sb")
        nc.vector.tensor_copy(oh_sb[:], oh_ps[0:4, 0:16])

        # ---------- class feat: cat[origin, origin - proto[cat]] ----------
        neg_ppad = ap.tile([4, 128], f32, tag="npp")
        nc.vector.tensor_scalar(neg_ppad[:], proto_pad_t[:], -1.0, None, op0=ALU.mult)
        cat2_ps = pt.tile([128, 16], f32, tag="tail")
        nc.tensor.matmul(cat2_ps[:], catid_t[:], origin[:], start=True, stop=False)
        nc.tensor.matmul(cat2_ps[:], neg_ppad[:], oh_sb[:], start=False, stop=True)
        cat2 = ap.tile([128, 16], f32, tag="cat2")
        nc.scalar.copy(cat2[:], cat2_ps[:])

        c1ps = pt.tile([128, 16], f32, tag="tail")
        nc.tensor.matmul(c1ps[:64, :], cw1_t[:, 0:64], cat2[:], start=True, stop=True)
        c1 = ap.tile([64, 16], f32, tag="c1")
        leaky(c1[:], c1ps[:64, :], 64, "c1")
        cf = ap.tile([64, 16], f32, tag="cf")
        c2ps = pt.tile([128, 16], f32, tag="tail")
        nc.tensor.matmul(c2ps[:64, :], cw2_t[:, 0:64], c1[:], start=True, stop=True)
        leaky(cf[:], c2ps[:64, :], 64, "cf")
        csvdd = ap.tile([1, 16], f32, tag="csvdd")
        svdd(cf[:], csvdd[:], "c")

        # ---------- align + final partials ----------
        al = ap.tile([1, 16], f32, tag="al")
        nc.vector.tensor_tensor(al[:], osvdd[:], csvdd[:], op=ALU.subtract)
        nc.scalar.activation(al[:], al[:], AF.Abs)

        outv = ap.tile([1, 4], f32, tag="outv")
        ce_ps = pt.tile([128, 16], f32, tag="tail")
        nc.tensor.matmul(ce_ps[0:1, 0:1], ce_col[:], ones16[:], start=True, stop=True)
        nc.vector.tensor_copy(outv[0:1, 0:1], ce_ps[0:1, 0:1])
        nc.vector.reduce_sum(outv[0:1, 1:2], osvdd[:], axis=AX.X)
        nc.vector.reduce_sum(outv[0:1, 2:3], csvdd[:], axis=AX.X)
        nc.vector.reduce_sum(outv[0:1, 3:4], al[:], axis=AX.X)
        nc.sync.dma_start(out=out_d[:], in_=outv[:])

    nc.compile()
    return nc


def _get_program():
    if "nc" not in _CACHE:
        _CACHE["nc"] = _build_program()
    return _CACHE["nc"]


def _host_prep(inputs):
    f = np.float32
    xm = np.ascontiguousarray(np.asarray(inputs["x_mid"], f).reshape(B, 512, 784))
    xd = np.ascontiguousarray(np.asarray(inputs["x_deep"], f).reshape(B, 100352))

    def T(w):
        return np.ascontiguousarray(np.asarray(w, f).T)

    # ow1.T with rows permuted so row j*128+p corresponds to feature d=16p+j,
    # matching the x_deep pooled layout.
    ow1T_nat = T(inputs["ow1"])  # [2048, 1024]
    ow1T = np.ascontiguousarray(
        ow1T_nat.reshape(128, 16, 1024).transpose(1, 0, 2).reshape(2048, 1024))

    center = np.asarray(inputs["center"], f)
    proto = np.asarray(inputs["proto"], f)
    proto_pad = np.zeros((4, 128), f)
    proto_pad[:, 64:] = proto
    center_pad = np.zeros((1, 128), f)
    center_pad[0, 64:] = center
    catid = np.zeros((64, 128), f)
    catid[np.arange(64), np.arange(64)] = 1
    catid[np.arange(64), 64 + np.arange(64)] = 1

    shared = {
        "wshT": T(inputs["w_shallow"]).astype(BF),
        "ow1T": ow1T.astype(BF),
        "sw1T": T(inputs["sw1"]).astype(BF),
        "ow2T": T(inputs["ow2"]).astype(BF),
        "sw2T": T(inputs["sw2"]).astype(BF),
        "ow3T": T(inputs["ow3"]).astype(BF),
        "sw3T": T(inputs["sw3"]).astype(BF),
        "tw1T": T(inputs["tw1"]),
        "tw2T": T(inputs["tw2"]),
        "cw1T": T(inputs["cw1"]),
        "cw2T": T(inputs["cw2"]),
        "qw1T": T(inputs["qw1"]),
        "qw2T": T(inputs["qw2"]),
        "protoT": T(proto),
        "proto_pad": proto_pad,
        "center_pad": center_pad,
        "center_col": np.ascontiguousarray(center.reshape(64, 1)),
        "catid": catid,
        "id16": np.eye(16, dtype=f),
    }
    in_maps = []
    for c in range(N_CORES):
        m = dict(shared)
        m["xm"] = np.ascontiguousarray(xm[c * BC:(c + 1) * BC])
        m["xd"] = np.ascontiguousarray(xd[c * BC:(c + 1) * BC])
        in_maps.append(m)
    return in_maps


def _combine(parts):
    tot = np.sum([np.asarray(p, np.float64).ravel() for p in parts], axis=0)
    return (tot / B).astype(np.float32).reshape(4, 1)


def _run(inputs, trace=False):
    from concourse.bass_utils import run_bass_kernel_spmd
    nc = _get_program()
    in_maps = _host_prep(inputs)
    kw = {}
    if trace:
        kw = dict(trace=True, trace_cores=list(range(N_CORES)))
    res = run_bass_kernel_spmd(nc, in_maps, list(range(N_CORES)), **kw)
    out = _combine([res.results[i]["out"] for i in range(N_CORES)])
    return out, res


def kernel(**inputs):
    out, _ = _run(inputs, trace=False)
    return out


def kernel_traced(**inputs):
    """Returns (output, exec_time_ns) using the NTFF profile (max over cores)."""
    out, res = _run(inputs, trace=True)
    return out, res.exec_time_ns


# revision 3
# speedup vs baseline: 1.1227x; 1.1227x over previous
"""DGAD net (vq_codebook) kernel for 8x Trainium2 NeuronCores.

Contract: kernel(**inputs) takes the FULL unsharded inputs (numpy, keyed as in
setup_inputs) and returns the FULL [4,1] float32 output. Inside, the batch
(128) is sharded 16-per-core across 8 cores (data parallel); weights are
replicated. Each core emits [1,4] partial sums (ce, origin_svdd, class_svdd,
align); the final all-reduce (sum across 8 cores, /128) happens on host during
the unshard step.

Key algebraic move: mean_HW(einsum('bchw,oc->bohw', x, w)) ==
einsum('bc,oc->bo', mean_HW(x), w) — the 2e11-FLOP 1x1 conv collapses to
pooling x_mid (205MB of streaming, the real cost) plus a tiny matmul.

Per-core pipeline:
  - x_deep [16,2048,49] pooled over HW with one 6.4MB DMA + one DVE reduce.
    The flat [16, 100352] view is split p=128 x 784 so DMA runs are 3136B
    contiguous; feature d lives at (partition p, col j) with d = 16p + j, and
    ow1's rows are pre-permuted on host to match.
  - x_mid [16,512,784] streamed in 8 3.2MB chunks (c on partitions, HW
    contiguous), DVE reduce_sum per chunk.
  - MLP chains on the PE in bf16 (host pre-transposes + casts weights; fp32
    PSUM accumulation). leaky_relu = max(x, 0.01x) via ACT mul + DVE max.
  - VQ tail in fp32: sim[b,k] = |t|^2 - 2 t.p + |p|^2 via one augmented
    matmul; argmax via is_ge(sim, rowmax) one-hot; proto gather via one-hot
    matmul; CE = log(sum exp(sim - max)); svdd partition-reductions via
    ones-vector matmuls.
"""

import numpy as np
import ml_dtypes

N_CORES = 8
B = 128
BC = B // N_CORES  # 16 samples per core

BF = ml_dtypes.bfloat16
F8 = ml_dtypes.float8_e4m3
WSCALE = 256.0  # fp8 weights stored *256; 1/256 folded into consumer scales

_CACHE = {}


def _build_program():
    import concourse.bass as bass  # noqa: F401
    import concourse.mybir as mybir
    import concourse.tile as tile
    from concourse import bacc
    from contextlib import ExitStack

    dt = mybir.dt
    AF = mybir.ActivationFunctionType
    ALU = mybir.AluOpType
    AX = mybir.AxisListType
    f32, bf16, f8 = dt.float32, dt.bfloat16, dt.float8e4
    INV = 1.0 / WSCALE

    nc = bacc.Bacc("TRN2", target_bir_lowering=False, debug=False,
                   enable_asserts=True, num_devices=N_CORES)

    def din(name, shape, d):
        return nc.dram_tensor(name, shape, d, kind="ExternalInput").ap()

    xm = din("xm", [BC, 512, 784], f32)
    xd = din("xd", [BC, 100352], f32)
    wshT_d = din("wshT", [512, 2048], f8)
    ow1T_d = din("ow1T", [2048, 1024], f8)  # rows pre-permuted: row j*128+p = ow1.T row 16p+j
    sw1T_d = din("sw1T", [2048, 1024], f8)
    ow2T_d = din("ow2T", [1024, 512], f8)
    sw2T_d = din("sw2T", [1024, 512], f8)
    ow3T_d = din("ow3T", [512, 64], f8)
    sw3T_d = din("sw3T", [512, 64], f8)
    tw1T_d = din("tw1T", [128, 64], f32)
    tw2T_d = din("tw2T", [64, 64], f32)
    cw1T_d = din("cw1T", [128, 64], f32)
    cw2T_d = din("cw2T", [64, 64], f32)
    qw1T_d = din("qw1T", [64, 64], f32)
    qw2T_d = din("qw2T", [64, 64], f32)
    protoT_d = din("protoT", [64, 4], f32)
    proto_pad_d = din("proto_pad", [4, 128], f32)
    center_pad_d = din("center_pad", [1, 128], f32)
    center_col_d = din("center_col", [64, 1], f32)
    catid_d = din("catid", [64, 128], f32)
    id16_d = din("id16", [16, 16], f32)
    out_d = nc.dram_tensor("out", [1, 4], f32, kind="ExternalOutput").ap()

    with tile.TileContext(nc) as tc, ExitStack() as ctx:
        wp = ctx.enter_context(tc.tile_pool(name="wp", bufs=1))
        xp = ctx.enter_context(tc.tile_pool(name="xp", bufs=3))
        dp = ctx.enter_context(tc.tile_pool(name="dp", bufs=1))
        ap = ctx.enter_context(tc.tile_pool(name="ap", bufs=1))
        pp = ctx.enter_context(tc.tile_pool(name="pp", bufs=4, space="PSUM"))
        pt = ctx.enter_context(tc.tile_pool(name="pt", bufs=3, space="PSUM"))
        sp_pool = ctx.enter_context(tc.tile_pool(name="spp", bufs=1, space="PSUM"))

        # ---------- ACT table preload (Lrelu/Exp/Ln) ----------
        scr = ap.tile([1, 1], f32, tag="scr")
        scr2 = ap.tile([1, 1], f32, tag="scr2")
        nc.gpsimd.memset(scr[:], 1.0)
        nc.scalar.activation(scr2[:], scr[:], AF.Lrelu, alpha=0.01)
        nc.scalar.activation(scr2[:], scr[:], AF.Exp)
        nc.scalar.activation(scr2[:], scr[:], AF.Ln)

        # ---------- DMAs: xd first, then weights/consts, xm stream below ----------
        xd_t = dp.tile([128, 16, 16, 49], f32, tag="xd")
        nc.sync.dma_start(out=xd_t[:],
                          in_=xd.rearrange("b (p j h) -> p b j h", p=128, j=16, h=49))

        ow1_t = wp.tile([128, 16, 1024], f8, tag="ow1")
        nc.sync.dma_start(out=ow1_t[:], in_=ow1T_d.rearrange("(j p) o -> p j o", p=128))

        wsh_t = wp.tile([128, 4, 2048], f8, tag="wsh")
        sw1_t = wp.tile([128, 16, 1024], f8, tag="sw1")
        ow2_t = wp.tile([128, 8, 512], f8, tag="ow2")
        sw2_t = wp.tile([128, 8, 512], f8, tag="sw2")
        ow3_t = wp.tile([128, 4, 64], f8, tag="ow3")
        sw3_t = wp.tile([128, 4, 64], f8, tag="sw3")
        tw1_t = wp.tile([128, 64], f32, tag="tw1")
        tw2_t = wp.tile([64, 64], f32, tag="tw2")
        cw1_t = wp.tile([128, 64], f32, tag="cw1")
        cw2_t = wp.tile([64, 64], f32, tag="cw2")
        qw1_t = wp.tile([64, 64], f32, tag="qw1")
        qw2_t = wp.tile([64, 64], f32, tag="qw2")
        protoT_t = wp.tile([64, 4], f32, tag="protoT")
        proto_pad_t = wp.tile([4, 128], f32, tag="proto_pad")
        center_pad_t = wp.tile([1, 128], f32, tag="center_pad")
        center_col_t = wp.tile([64, 1], f32, tag="center_col")
        catid_t = wp.tile([64, 128], f32, tag="catid")
        id16_t = wp.tile([16, 16], f32, tag="id16")

        # small consts early (cheap; lets tail prep run during the stream)
        for t_, d_ in ((tw1_t, tw1T_d), (tw2_t, tw2T_d), (cw1_t, cw1T_d),
                       (cw2_t, cw2T_d), (qw1_t, qw1T_d), (qw2_t, qw2T_d),
                       (protoT_t, protoT_d), (proto_pad_t, proto_pad_d),
                       (center_pad_t, center_pad_d), (center_col_t, center_col_d),
                       (catid_t, catid_d), (id16_t, id16_d)):
            nc.sync.dma_start(out=t_[:], in_=d_)
        nc.sync.dma_start(out=ow2_t[:], in_=ow2T_d.rearrange("(k p) o -> p k o", p=128))
        nc.sync.dma_start(out=sw2_t[:], in_=sw2T_d.rearrange("(k p) o -> p k o", p=128))
        nc.sync.dma_start(out=ow3_t[:], in_=ow3T_d.rearrange("(k p) o -> p k o", p=128))
        nc.sync.dma_start(out=sw3_t[:], in_=sw3T_d.rearrange("(k p) o -> p k o", p=128))

        # ---------- x_deep pool ----------
        xdsum = ap.tile([128, 16, 16], f32, tag="xdsum")
        nc.vector.reduce_sum(xdsum[:], xd_t[:], axis=AX.X)
        xdb = ap.tile([128, 16, 16], bf16, tag="xdb")
        nc.vector.tensor_scalar(xdb[:], xdsum[:], INV / 49.0, None, op0=ALU.mult)

        # ---------- hoisted const-derived prep ----------
        ones64 = ap.tile([64, 1], f32, tag="ones64")
        nc.gpsimd.memset(ones64[:], 1.0)
        ones16 = ap.tile([16, 1], f32, tag="ones16")
        nc.gpsimd.memset(ones16[:], 1.0)
        ones1x16 = ap.tile([1, 16], f32, tag="ones1x16")
        nc.gpsimd.memset(ones1x16[:], 1.0)
        neg_center_pad = ap.tile([1, 128], f32, tag="ncp")
        nc.vector.tensor_scalar(neg_center_pad[:], center_pad_t[:], -1.0, None, op0=ALU.mult)
        neg_ppad = ap.tile([4, 128], f32, tag="npp")
        nc.vector.tensor_scalar(neg_ppad[:], proto_pad_t[:], -1.0, None, op0=ALU.mult)
        rhs_sim = ap.tile([65, 4], f32, tag="rhs_sim")
        nc.vector.tensor_scalar(rhs_sim[0:64, :], protoT_t[:], -2.0, None, op0=ALU.mult)
        nc.gpsimd.memset(rhs_sim[64:65, :], 1.0)
        pT2 = ap.tile([64, 4], f32, tag="pT2")
        nc.vector.tensor_tensor(pT2[:], protoT_t[:], protoT_t[:], op=ALU.mult)
        pn_ps = pt.tile([128, 16], f32, tag="tail")
        nc.tensor.matmul(pn_ps[0:1, 0:4], ones64[:], pT2[:], start=True, stop=True)
        pnorm = ap.tile([1, 4], f32, tag="pnorm")
        nc.scalar.copy(pnorm[:], pn_ps[0:1, 0:4])

        # generic chain layer; act_scale folds the fp8 weight prescale
        def layer(w_t, n_k, n_m, m_sz, rhs_fn, dst_fn, act_scale, act=True):
            for m in range(n_m):
                ps = pp.tile([128, 16], f32, tag="mm")
                for k in range(n_k):
                    nc.tensor.matmul(ps[:m_sz, :], w_t[:, k, m * m_sz:(m + 1) * m_sz],
                                     rhs_fn(k), start=(k == 0), stop=(k == n_k - 1))
                if act:
                    nc.scalar.activation(dst_fn(m), ps[:m_sz, :], AF.Lrelu,
                                         scale=act_scale, alpha=0.01)
                else:
                    nc.scalar.mul(dst_fn(m), ps[:m_sz, :], act_scale)

        # ---------- origin chain (rhs already carries 1/256) ----------
        y1o = ap.tile([128, 8, 16], bf16, tag="y1o")
        layer(ow1_t, 16, 8, 128, lambda k: xdb[:, :, k], lambda m: y1o[:, m, :], 1.0)
        y2o = ap.tile([128, 4, 16], bf16, tag="y2o")
        layer(ow2_t, 8, 4, 128, lambda k: y1o[:, k, :], lambda m: y2o[:, m, :], INV)
        origin = ap.tile([64, 16], f32, tag="origin")
        layer(ow3_t, 4, 1, 64, lambda k: y2o[:, k, :], lambda m: origin[:], INV)

        # ---------- qw chain + origin_svdd ----------
        def small_mlp(wa, wb, rhs, dst, tagn):
            psa = pt.tile([128, 16], f32, tag="tail")
            nc.tensor.matmul(psa[:64, :], wa[:, 0:64], rhs, start=True, stop=True)
            mid = ap.tile([64, 16], f32, tag="mid_" + tagn)
            nc.scalar.activation(mid[:], psa[:64, :], AF.Lrelu, alpha=0.01)
            psb = pt.tile([128, 16], f32, tag="tail")
            nc.tensor.matmul(psb[:64, :], wb[:, 0:64], mid[:], start=True, stop=True)
            nc.scalar.activation(dst, psb[:64, :], AF.Lrelu, alpha=0.01)

        def svdd(feat, dst_sb, tagn):
            d_ = ap.tile([64, 16], f32, tag="d_" + tagn)
            nc.vector.tensor_scalar(d_[:], feat, center_col_t[:, 0:1], None, op0=ALU.subtract)
            sq = ap.tile([64, 16], f32, tag="sq_" + tagn)
            nc.vector.tensor_tensor(sq[:], d_[:], d_[:], op=ALU.mult)
            psv = pt.tile([128, 16], f32, tag="tail")
            nc.tensor.matmul(psv[0:1, :], ones64[:], sq[:], start=True, stop=True)
            nc.scalar.copy(dst_sb, psv[0:1, :])

        qf = ap.tile([64, 16], f32, tag="qf")
        small_mlp(qw1_t, qw2_t, origin[:], qf[:], "q")
        osvdd = ap.tile([1, 16], f32, tag="osvdd")
        svdd(qf[:], osvdd[:], "o")

        # ---------- x_mid stream: 16 quarter-chunks + in-stream shallow ----------
        xmsum = ap.tile([128, 4, 16], f32, tag="xmsum")
        xmb = ap.tile([128, 4, 16], bf16, tag="xmb")
        sp_ps = sp_pool.tile([128, 256], f32, tag="sp")
        for q in range(16):
            cc, sub = divmod(q, 4)
            b0 = 4 * sub
            t = xp.tile([128, 4, 784], f32, tag="xmt")
            nc.sync.dma_start(
                out=t[:],
                in_=xm[b0:b0 + 4, cc * 128:(cc + 1) * 128, :].rearrange("b c h -> c b h"))
            nc.vector.reduce_sum(xmsum[:, cc, b0:b0 + 4], t[:], axis=AX.X)
            if q == 0:
                nc.sync.dma_start(out=wsh_t[:], in_=wshT_d.rearrange("(c p) o -> p c o", p=128))
            elif q == 1:
                nc.sync.dma_start(out=sw1_t[:], in_=sw1T_d.rearrange("(k p) o -> p k o", p=128))
            if sub == 3:
                nc.vector.tensor_scalar(xmb[:, cc, :], xmsum[:, cc, :], INV / 784.0,
                                        None, op0=ALU.mult)
                for m in range(16):
                    nc.tensor.matmul(sp_ps[:, 16 * m:16 * m + 16],
                                     wsh_t[:, cc, m * 128:(m + 1) * 128],
                                     xmb[:, cc, :], start=(cc == 0), stop=(cc == 3))

        spb = ap.tile([128, 16, 16], bf16, tag="spb")
        nc.scalar.copy(spb[:], sp_ps[:])

        # ---------- shallow chain ----------
        y1s = ap.tile([128, 8, 16], bf16, tag="y1s")
        layer(sw1_t, 16, 8, 128, lambda k: spb[:, k, :], lambda m: y1s[:, m, :], INV)
        y2s = ap.tile([128, 4, 16], bf16, tag="y2s")
        layer(sw2_t, 8, 4, 128, lambda k: y1s[:, k, :], lambda m: y2s[:, m, :], INV)
        shallow = ap.tile([64, 16], f32, tag="shallow")
        layer(sw3_t, 4, 1, 64, lambda k: y2s[:, k, :], lambda m: shallow[:], INV)

        # ---------- texture ----------
        cat1_ps = pt.tile([128, 16], f32, tag="tail")
        nc.tensor.matmul(cat1_ps[:], catid_t[:], shallow[:], start=True, stop=False)
        nc.tensor.matmul(cat1_ps[:], neg_center_pad[:], ones1x16[:], start=False, stop=True)
        cat1 = ap.tile([128, 16], f32, tag="cat1")
        nc.scalar.copy(cat1[:], cat1_ps[:])

        t1ps = pt.tile([128, 16], f32, tag="tail")
        nc.tensor.matmul(t1ps[:64, :], tw1_t[:, 0:64], cat1[:], start=True, stop=True)
        t1 = ap.tile([64, 16], f32, tag="t1")
        nc.scalar.activation(t1[:], t1ps[:64, :], AF.Lrelu, alpha=0.01)
        sim_lhs = ap.tile([65, 16], f32, tag="sim_lhs")
        t2ps = pt.tile([128, 16], f32, tag="tail")
        nc.tensor.matmul(t2ps[:64, :], tw2_t[:, 0:64], t1[:], start=True, stop=True)
        nc.scalar.activation(sim_lhs[0:64, :], t2ps[:64, :], AF.Lrelu, alpha=0.01)

        # ---------- sim + CE + argmax ----------
        t2 = ap.tile([64, 16], f32, tag="t2")
        nc.vector.tensor_tensor(t2[:], sim_lhs[0:64, :], sim_lhs[0:64, :], op=ALU.mult)
        tsq_ps = pt.tile([128, 16], f32, tag="tail")
        nc.tensor.matmul(tsq_ps[0:1, :], ones64[:], t2[:], start=True, stop=True)
        nc.scalar.copy(sim_lhs[64:65, :], tsq_ps[0:1, :])

        sim_ps = pt.tile([128, 16], f32, tag="tail")
        nc.tensor.matmul(sim_ps[0:16, 0:4], sim_lhs[:], rhs_sim[:], start=True, stop=False)
        nc.tensor.matmul(sim_ps[0:16, 0:4], ones1x16[:], pnorm[:], start=False, stop=True)
        sim_sb = ap.tile([16, 4], f32, tag="sim_sb")
        nc.vector.tensor_copy(sim_sb[:], sim_ps[0:16, 0:4])

        m16 = ap.tile([16, 1], f32, tag="m16")
        nc.vector.reduce_max(m16[:], sim_sb[:], axis=AX.X)
        negm = ap.tile([16, 1], f32, tag="negm")
        nc.vector.reduce_max(negm[:], sim_sb[:], axis=AX.X, negate=True)
        e_t = ap.tile([16, 4], f32, tag="e_t")
        s16 = ap.tile([16, 1], f32, tag="s16")
        nc.scalar.activation(e_t[:], sim_sb[:], AF.Exp, bias=negm[:, 0:1], accum_out=s16[:])
        ce_col = ap.tile([16, 1], f32, tag="ce_col")
        nc.scalar.activation(ce_col[:], s16[:], AF.Ln)

        onehotT = ap.tile([16, 4], f32, tag="onehotT")
        nc.vector.tensor_scalar(onehotT[:], sim_sb[:], m16[:, 0:1], None, op0=ALU.is_ge)
        oh_ps = pt.tile([128, 16], f32, tag="tail")
        nc.tensor.transpose(oh_ps[0:4, 0:16], onehotT[:], id16_t[:])
        oh_sb = ap.tile([4, 16], f32, tag="oh_sb")
        nc.vector.tensor_copy(oh_sb[:], oh_ps[0:4, 0:16])

        # ---------- class feat chain ----------
        cat2_ps = pt.tile([128, 16], f32, tag="tail")
        nc.tensor.matmul(cat2_ps[:], catid_t[:], origin[:], start=True, stop=False)
        nc.tensor.matmul(cat2_ps[:], neg_ppad[:], oh_sb[:], start=False, stop=True)
        cat2 = ap.tile([128, 16], f32, tag="cat2")
        nc.scalar.copy(cat2[:], cat2_ps[:])

        cf = ap.tile([64, 16], f32, tag="cf")
        small_mlp(cw1_t, cw2_t, cat2[:], cf[:], "c")
        csvdd = ap.tile([1, 16], f32, tag="csvdd")
        svdd(cf[:], csvdd[:], "c")

        # ---------- align + partials ----------
        al = ap.tile([1, 16], f32, tag="al")
        nc.vector.tensor_tensor(al[:], osvdd[:], csvdd[:], op=ALU.subtract)
        nc.scalar.activation(al[:], al[:], AF.Abs)

        outv = ap.tile([1, 4], f32, tag="outv")
        ce_ps = pt.tile([128, 16], f32, tag="tail")
        nc.tensor.matmul(ce_ps[0:1, 0:1], ce_col[:], ones16[:], start=True, stop=True)
        nc.vector.tensor_copy(outv[0:1, 0:1], ce_ps[0:1, 0:1])
        nc.vector.reduce_sum(outv[0:1, 1:2], osvdd[:], axis=AX.X)
        nc.vector.reduce_sum(outv[0:1, 2:3], csvdd[:], axis=AX.X)
        nc.vector.reduce_sum(outv[0:1, 3:4], al[:], axis=AX.X)
        nc.sync.dma_start(out=out_d[:], in_=outv[:])

    nc.compile()
    return nc



def _host_prep(inputs):
    f = np.float32
    xm = np.ascontiguousarray(np.asarray(inputs["x_mid"], f).reshape(B, 512, 784))
    xd = np.ascontiguousarray(np.asarray(inputs["x_deep"], f).reshape(B, 100352))

    def T(w):
        return np.ascontiguousarray(np.asarray(w, f).T)

    def T8(w):
        return (T(w) * WSCALE).astype(F8)

    ow1T_nat = T(inputs["ow1"])
    ow1T = np.ascontiguousarray(
        ow1T_nat.reshape(128, 16, 1024).transpose(1, 0, 2).reshape(2048, 1024))

    center = np.asarray(inputs["center"], f)
    proto = np.asarray(inputs["proto"], f)
    proto_pad = np.zeros((4, 128), f)
    proto_pad[:, 64:] = proto
    center_pad = np.zeros((1, 128), f)
    center_pad[0, 64:] = center
    catid = np.zeros((64, 128), f)
    catid[np.arange(64), np.arange(64)] = 1
    catid[np.arange(64), 64 + np.arange(64)] = 1

    shared = {
        "wshT": T8(inputs["w_shallow"]),
        "ow1T": (ow1T * WSCALE).astype(F8),
        "sw1T": T8(inputs["sw1"]),
        "ow2T": T8(inputs["ow2"]),
        "sw2T": T8(inputs["sw2"]),
        "ow3T": T8(inputs["ow3"]),
        "sw3T": T8(inputs["sw3"]),
        "tw1T": T(inputs["tw1"]),
        "tw2T": T(inputs["tw2"]),
        "cw1T": T(inputs["cw1"]),
        "cw2T": T(inputs["cw2"]),
        "qw1T": T(inputs["qw1"]),
        "qw2T": T(inputs["qw2"]),
        "protoT": T(proto),
        "proto_pad": proto_pad,
        "center_pad": center_pad,
        "center_col": np.ascontiguousarray(center.reshape(64, 1)),
        "catid": catid,
        "id16": np.eye(16, dtype=f),
    }
    in_maps = []
    for c in range(N_CORES):
        m = dict(shared)
        m["xm"] = np.ascontiguousarray(xm[c * BC:(c + 1) * BC])
        m["xd"] = np.ascontiguousarray(xd[c * BC:(c + 1) * BC])
        in_maps.append(m)
    return in_maps



def _get_program():
    if "nc" not in _CACHE:
        _CACHE["nc"] = _build_program()
    return _CACHE["nc"]


def _combine(parts):
    tot = np.sum([np.asarray(p, np.float64).ravel() for p in parts], axis=0)
    return (tot / B).astype(np.float32).reshape(4, 1)


def _run(inputs, trace=False):
    from concourse.bass_utils import run_bass_kernel_spmd
    nc = _get_program()
    in_maps = _host_prep(inputs)
    kw = {}
    if trace:
        kw = dict(trace=True, trace_cores=list(range(N_CORES)))
    res = run_bass_kernel_spmd(nc, in_maps, list(range(N_CORES)), **kw)
    out = _combine([res.results[i]["out"] for i in range(N_CORES)])
    return out, res


def kernel(**inputs):
    out, _ = _run(inputs, trace=False)
    return out


def kernel_traced(**inputs):
    """Returns (output, exec_time_ns) using the NTFF profile (max over cores)."""
    out, res = _run(inputs, trace=True)
    return out, res.exec_time_ns
